# revision 1
# baseline (speedup 1.0000x reference)
"""Trainium2 Bass kernel for BlockAttnRes.compute_all_inputs.

Math: for each row (b,t), layer l attends over a small per-row source stack
(embedding, completed block sums S_k, and the running partial sum). Every
source is a prefix-sum of the 25 "raw" per-row vectors X = [emb, f_0..f_23],
i.e. sources V = M @ X for a constant 0/1 matrix M (25x25). Likewise the
output h_l = sum_n alpha_{l,n} v_n = (A M) @ X, and the score dots
v_n . qw_l = M @ (X @ qw^T). So the whole layer loop collapses into a few
small matmuls per row batch - no sequential layer recurrence on device.

Device layout: batches of R=5 rows; partition p = r*25 + j (r-major), j in
[0, 25) raw index, so P = 125 partitions. Inputs are host-transposed to
[row, j, d] so each batch loads with ONE contiguous DMA; the output is
written [row, l, d] and host-transposed back. Per batch:
  1. DMA X [125, 2048] fp32 (1MB contiguous)
  2. PE transposes X chunks -> X^T (fp32), ACT copies to SBUF as bf16
  3. PE: per d-chunk matmul lhsT=X^T_chunk rhs=[X^T_chunk | qw^T_chunk]
     accumulating SC = [Gram | G_X] (bf16 inputs, fp32 accum)
  4. PE: M-fold: Mout = MT_bd.T @ SC = [v_n.x_j' | v_n.qw_l]
  5. DVE: sumsq_n = sum_j'(masked Mout); ACT: rsqrt via exp(-0.5*ln(x))
  6. scores scaled, transposed, masked softmax over sources (tiny ops)
  7. alphas folded through M (PE) -> B^T, H = B^T.T @ X in fp32r
  8. H PSUM -> SBUF -> one contiguous DMA out

Sharding: data-parallel over B*T = 2048 rows -> 8 cores x 256 rows.
"""

import numpy as np
import ml_dtypes

import concourse.bass as bass
import concourse.bacc as bacc
import concourse.mybir as mybir
from concourse import tile
from concourse.alu_op_type import AluOpType
from concourse.bass_utils import run_bass_kernel_spmd

L = 24
D = 2048
NUM_BLOCKS = 8
EPS = 1e-6
B, T = 2, 1024
N_CORES = 8

ROWS_PER_CORE = (B * T) // N_CORES  # 256
R = 5            # rows per batch
NJ = 25          # raw vectors per row: emb + 24 layer outputs
NS = 25          # sources per row: emb + (C_k1, C_k2, S_k) x 8 blocks
P = NJ * R       # 125 partitions per batch
NCHUNK = D // 128  # 16 d-chunks
CW = 152         # xt_sb column stride per chunk: 125 X^T + 24 qw + 3 pad
SCW = P + L      # 149 = gram + score columns
XF = D + 32      # padded row pitch (avoids flat-merged partition APs)
NEG = -1e30

f32 = mybir.dt.float32
f32r = mybir.dt.float32r
bf16 = mybir.dt.bfloat16


def _source_matrix():
    """M[n, j]: source n = sum_j M[n,j] * raw_j. Raw j=0 is emb, j=1+l is f_l.
    Sources: n=0 emb; n=1+3k+i (i=0,1,2) is C_{k,i+1} = f_{3k}+..+f_{3k+i}."""
    M = np.zeros((NS, NJ), dtype=np.float32)
    M[0, 0] = 1.0
    for k in range(NUM_BLOCKS):
        for i in range(3):
            n = 1 + 3 * k + i
            M[n, 1 + 3 * k : 1 + 3 * k + i + 1] = 1.0
    return M


def _valid_matrix():
    """valid[l, n]: which sources layer l attends over (block k=l//3, i=l%3):
    emb; S_k (n=3k+3) for k < l//3; partial C_{l//3, i} (n = 3*(l//3)+i) if i>0."""
    V = np.zeros((L, NS), dtype=bool)
    for l in range(L):
        kb, ii = l // 3, l % 3
        V[l, 0] = True
        for k in range(kb):
            V[l, 3 * k + 3] = True
        if ii > 0:
            V[l, 3 * kb + ii] = True
    return V


def _build_consts(queries, key_norm_weight):
    M = _source_matrix()
    valid = _valid_matrix()
    eye_r = np.eye(R, dtype=np.float32)

    qw = (queries * key_norm_weight[None, :]).astype(np.float32)  # [L, D]
    # qwT[p, c*24 + l] = qw[l, c*128 + p]
    qwT = np.ascontiguousarray(
        qw.reshape(L, NCHUNK, 128).transpose(2, 1, 0).reshape(128, NCHUNK * L)
    ).astype(ml_dtypes.bfloat16)

    # mtbd[(r,j),(r',n)] = (r==r') * M[n,j]   (lhsT of the M-fold matmul)
    mtbd = np.einsum("nj,ab->ajbn", M, eye_r).reshape(P, NS * R)
    mtbd = np.ascontiguousarray(mtbd).astype(ml_dtypes.bfloat16)
    # mbd[(r,n),(r',j)] = (r==r') * M[n,j]    (sumsq mask + B-fold lhsT)
    mbd = np.einsum("nj,ab->anbj", M, eye_r).reshape(NS * R, P)
    mbd = np.ascontiguousarray(mbd).astype(np.float32)
    # diagm[(r,n),(r',l)] = (r==r')
    diagm = np.einsum("ab,nl->anbl", eye_r, np.ones((NS, L), np.float32))
    diagm = np.ascontiguousarray(diagm.reshape(P, R * L)).astype(np.float32)
    # maskneg[l, (r,n)] = 0 if valid else NEG
    maskneg = np.where(valid[:, None, :], 0.0, NEG)  # [L, 1, NS] -> bcast r
    maskneg = np.broadcast_to(maskneg, (L, R, NS)).reshape(L, R * NS)
    maskneg = np.ascontiguousarray(maskneg).astype(np.float32)

    ident = np.eye(128, dtype=np.float32)
    return dict(qwT=qwT, mtbd=mtbd, mbd=mbd, diagm=diagm, maskneg=maskneg,
                ident=ident)


def _batch_starts():
    starts = [R * b for b in range(ROWS_PER_CORE // R)]  # 0..250
    if starts[-1] + R < ROWS_PER_CORE:
        starts.append(ROWS_PER_CORE - R)  # 251 (overlaps; identical rewrites)
    return starts


def build_kernel():
    nc = bacc.Bacc("TRN2", target_bir_lowering=False, debug=False)

    # host-transposed input: row-major [row, j, d] flattened. Declared f32r
    # (same bits as fp32) so the PE can consume it at full rate; walrus
    # requires fp32r matmul operands to be produced as fp32r. Row pitch is
    # padded to XF so the HBM-side read AP cannot flat-merge: unmerged per-row
    # descriptors spread the load DMA across all 16 SDMA engines (a merged
    # contiguous read is chunked into ~5 big descriptors = 5 engines only).
    loT = nc.dram_tensor("loT", [ROWS_PER_CORE * NJ, XF], f32r,
                         kind="ExternalInput").ap()
    qwT_d = nc.dram_tensor("qwT", [128, NCHUNK * L], bf16, kind="ExternalInput").ap()
    mtbd_d = nc.dram_tensor("mtbd", [P, NS * R], bf16, kind="ExternalInput").ap()
    mbd_d = nc.dram_tensor("mbd", [NS * R, P], f32, kind="ExternalInput").ap()
    diagm_d = nc.dram_tensor("diagm", [P, R * L], f32, kind="ExternalInput").ap()
    maskneg_d = nc.dram_tensor("maskneg", [L, R * NS], f32, kind="ExternalInput").ap()
    ident_d = nc.dram_tensor("ident", [128, 128], f32, kind="ExternalInput").ap()
    identr_d = nc.dram_tensor("identr", [128, 128], f32r, kind="ExternalInput").ap()
    # output [row, l, d] flattened; host transposes back to [l, row, d]
    outT = nc.dram_tensor("outT", [ROWS_PER_CORE * L, D], f32,
                          kind="ExternalOutput").ap()

    with tile.TileContext(nc) as tc:
        with (
            tc.tile_pool(name="const", bufs=1) as const,
            tc.tile_pool(name="xpool", bufs=4) as xpool,
            tc.tile_pool(name="xtpool", bufs=3) as xtpool,
            tc.tile_pool(name="scpool", bufs=3) as scpool,
            tc.tile_pool(name="hpool", bufs=3) as hpool,
            tc.tile_pool(name="small", bufs=2) as small,
            tc.tile_pool(name="ps_xt", bufs=3, space=bass.MemorySpace.PSUM) as ps_xt,
            tc.tile_pool(name="ps_sc", bufs=1, space=bass.MemorySpace.PSUM) as ps_sc,
            tc.tile_pool(name="ps_m", bufs=1, space=bass.MemorySpace.PSUM) as ps_m,
            tc.tile_pool(name="ps_sm", bufs=1, space=bass.MemorySpace.PSUM) as ps_sm,
            tc.tile_pool(name="ps_h", bufs=2, space=bass.MemorySpace.PSUM) as ps_h,
        ):
            qwT = const.tile([128, NCHUNK * L], bf16)
            nc.sync.dma_start(qwT[:], qwT_d[:])
            mtbd = const.tile([P, NS * R], bf16)
            nc.sync.dma_start(mtbd[:], mtbd_d[:])
            mbd = const.tile([NS * R, P], f32)
            nc.sync.dma_start(mbd[:], mbd_d[:])
            diagm = const.tile([P, R * L], f32)
            nc.sync.dma_start(diagm[:], diagm_d[:])
            maskneg = const.tile([L, R * NS], f32)
            nc.sync.dma_start(maskneg[:], maskneg_d[:])
            ident = const.tile([128, 128], f32)
            nc.sync.dma_start(ident[:], ident_d[:])
            identr = const.tile([128, 128], f32r)
            nc.sync.dma_start(identr[:], identr_d[:])
            epsb = const.tile([P, 1], f32)
            nc.vector.memset(epsb[:], EPS)

            for row0 in _batch_starts():
                # ---- X = [emb; f_0..f_23] per row: one 1MB DMA, 16-way split
                X = xpool.tile([P, XF], f32r)
                nc.sync.dma_start(
                    X[:, 0:D], loT[row0 * NJ : row0 * NJ + P, 0:D]
                )

                # ---- X^T via PE transposes; bf16 copies into xt_sb
                xt_sb = xtpool.tile([128, NCHUNK * CW], bf16)
                xt3 = xt_sb.rearrange("p (c w) -> p c w", w=CW)
                nc.vector.tensor_copy(
                    xt3[:, :, P : P + L],
                    qwT.rearrange("p (c w) -> p c w", w=L),
                )
                for half in range(4):
                    xtp = ps_xt.tile([128, 512], f32r)
                    for cc in range(4):
                        c = 4 * half + cc
                        # fp32r dst needs an even innermost count: write 126
                        # cols via a zero-padded identity slice [I | 0]
                        nc.tensor.transpose(
                            xtp[:, 128 * cc : 128 * cc + P + 1],
                            X[:, 128 * c : 128 * (c + 1)],
                            identr[:P, : P + 1],
                        )
                    nc.scalar.copy(
                        xt3[:, 4 * half : 4 * half + 4, 0:P],
                        xtp.rearrange("p (cc w) -> p cc w", w=128)[:, :, 0:P],
                    )

                # ---- SC = [Gram | G_X] accumulated over d-chunks (bf16)
                SC = ps_sc.tile([P, 152], f32)
                for c in range(NCHUNK):
                    base = CW * c
                    nc.tensor.matmul(
                        SC[:, 0:SCW],
                        xt_sb[:, base : base + P],
                        xt_sb[:, base : base + SCW],
                        start=(c == 0),
                        stop=(c == NCHUNK - 1),
                    )
                SC_sb = scpool.tile([P, 152], bf16)
                nc.scalar.copy(SC_sb[:, 0:SCW], SC[:, 0:SCW])

                # ---- M-fold: Mout = [v_n . x_j' | v_n . qw_l]
                Mout = ps_m.tile([P, 152], f32)
                nc.tensor.matmul(
                    Mout[:, 0:SCW], mtbd[:], SC_sb[:, 0:SCW], start=True, stop=True
                )

                # ---- sumsq_n = sum over j' in source-set (masked row sum)
                junk = small.tile([P, P], f32)
                sumsq = small.tile([P, 1], f32)
                nc.vector.scalar_tensor_tensor(
                    out=junk[:],
                    in0=Mout[:, 0:P],
                    scalar=1.0,
                    in1=mbd[:],
                    op0=AluOpType.mult,
                    op1=AluOpType.mult,
                    accum_out=sumsq[:],
                )
                # rsqrt(mean+eps) = exp(-0.5 * ln(sumsq/D + eps))
                lnu = small.tile([P, 1], f32)
                nc.scalar.activation(
                    lnu[:], sumsq[:], mybir.ActivationFunctionType.Ln,
                    bias=epsb[:], scale=1.0 / D,
                )
                rsq = small.tile([P, 1], f32)
                nc.scalar.activation(
                    rsq[:], lnu[:], mybir.ActivationFunctionType.Exp, scale=-0.5
                )
                scoresR = small.tile([P, L], f32)
                nc.scalar.activation(
                    scoresR[:], Mout[:, P:SCW],
                    mybir.ActivationFunctionType.Copy, scale=rsq[:],
                )

                # ---- masked softmax over sources (free axis), per (r, l)
                scoreT = ps_sm.tile([L, P], f32, tag="sm")
                nc.tensor.transpose(scoreT[:], scoresR[:], ident[:P, :P])
                smask = small.tile([L, P], f32)
                nc.vector.tensor_add(smask[:], scoreT[:], maskneg[:])
                esc = small.tile([L, P], f32)
                nc.scalar.activation(
                    esc[:], smask[:], mybir.ActivationFunctionType.Exp
                )
                ssum = small.tile([L, R], f32)
                nc.vector.reduce_sum(
                    ssum[:],
                    esc.rearrange("p (r n) -> p r n", r=R),
                    axis=mybir.AxisListType.X,
                )
                rec = small.tile([L, R], f32)
                nc.vector.reciprocal(rec[:], ssum[:])
                alpha = small.tile([L, P], f32)
                nc.vector.tensor_tensor(
                    alpha.rearrange("p (r n) -> p r n", r=R),
                    esc.rearrange("p (r n) -> p r n", r=R),
                    rec.unsqueeze(2).broadcast_to([L, R, NS]),
                    AluOpType.mult,
                )

                # ---- fold alphas through M: B^T = M_bd.T @ alpha_bd
                alphaT = ps_sm.tile([P, L], f32, tag="sm")
                nc.tensor.transpose(alphaT[:], alpha[:], ident[:L, :L])
                abd = small.tile([P, R * L], f32)
                nc.vector.scalar_tensor_tensor(
                    out=abd.rearrange("p (r l) -> p r l", r=R),
                    in0=alphaT.unsqueeze(1).broadcast_to([P, R, L]),
                    scalar=1.0,
                    in1=diagm.rearrange("p (r l) -> p r l", r=R),
                    op0=AluOpType.mult,
                    op1=AluOpType.mult,
                )
                BT = ps_sm.tile([P, R * L], f32, tag="sm")
                nc.tensor.matmul(BT[:], mbd[:], abd[:], start=True, stop=True)
                btsb = small.tile([P, R * L], f32r)
                nc.scalar.copy(btsb[:], BT[:])

                # ---- H = B^T.T @ X  (fp32r, full-rate at N=512)
                H_sb = hpool.tile([R * L, XF], f32)
                for nb in range(4):
                    Hp = ps_h.tile([R * L, 512], f32)
                    nc.tensor.matmul(
                        Hp[:],
                        btsb[:],
                        X[:, 512 * nb : 512 * (nb + 1)],
                        start=True,
                        stop=True,
                    )
                    if nb % 2 == 0:
                        nc.scalar.copy(H_sb[:, 512 * nb : 512 * (nb + 1)], Hp[:])
                    else:
                        nc.vector.tensor_copy(
                            H_sb[:, 512 * nb : 512 * (nb + 1)], Hp[:]
                        )

                # out-DMA on the ACT HWDGE ring: keeps the sync ring free for
                # input prefetch (no head-of-line wait on H completion)
                nc.scalar.dma_start(
                    outT[row0 * L : row0 * L + R * L, :], H_sb[:, 0:D]
                )

    # Pin Ln/Exp to the one table set containing both, so the compiled stream
    # has a single ACT table load instead of two reloads (~2.7us) per batch.
    # Set names/order (= act_func_set ids) are preserved; only the contents
    # steering the per-activation set choice are filtered.
    real_gat = bacc.get_activation_tables
    AF = mybir.ActivationFunctionType

    def gat_pinned(arch):
        out = {}
        for name, fns in real_gat(arch).items():
            if name == "natural_log_exp_and_others":
                out[name] = set(fns)
            else:
                out[name] = {f for f in fns if f not in (AF.Ln, AF.Exp)}
        return out

    bacc.get_activation_tables = gat_pinned
    try:
        nc.compile()
    finally:
        bacc.get_activation_tables = real_gat
    return nc


_NC_CACHE = None


def _prep_loT(layer_outputs, embedding):
    """[L,B,T,D]+[B,T,D] -> per-row stacks [B*T, 25, XF] (row-major,
    rows padded to the XF pitch)."""
    lo_flat = layer_outputs.reshape(L, B * T, D)
    emb_flat = embedding.reshape(B * T, D)
    loT = np.zeros((B * T, NJ, XF), dtype=np.float32)
    loT[:, 0, :D] = emb_flat
    loT[:, 1:, :D] = lo_flat.transpose(1, 0, 2)
    return loT


def kernel(layer_outputs, embedding, queries, key_norm_weight):
    global _NC_CACHE
    layer_outputs = np.asarray(layer_outputs, dtype=np.float32)
    embedding = np.asarray(embedding, dtype=np.float32)
    queries = np.asarray(queries, dtype=np.float32)
    key_norm_weight = np.asarray(key_norm_weight, dtype=np.float32)

    loT = _prep_loT(layer_outputs, embedding)
    consts = _build_consts(queries, key_norm_weight)

    if _NC_CACHE is None:
        _NC_CACHE = build_kernel()
    nc = _NC_CACHE

    in_maps = []
    for c in range(N_CORES):
        r0 = c * ROWS_PER_CORE
        in_maps.append({
            "loT": loT[r0 : r0 + ROWS_PER_CORE].reshape(ROWS_PER_CORE * NJ, XF),
            "qwT": consts["qwT"],
            "mtbd": consts["mtbd"],
            "mbd": consts["mbd"],
            "diagm": consts["diagm"],
            "maskneg": consts["maskneg"],
            "ident": consts["ident"],
            "identr": consts["ident"],
        })

    res = run_bass_kernel_spmd(nc, in_maps, core_ids=list(range(N_CORES)))

    full = np.empty((L, B * T, D), dtype=np.float32)
    for c in range(N_CORES):
        r0 = c * ROWS_PER_CORE
        outT = res.results[c]["outT"].reshape(ROWS_PER_CORE, L, D)
        full[:, r0 : r0 + ROWS_PER_CORE, :] = outT.transpose(1, 0, 2)
    return full.reshape(L, B, T, D)



# revision 2
# speedup vs baseline: 2.8437x; 2.8437x over previous
"""Trainium2 Bass kernel for BlockAttnRes.compute_all_inputs (bf16 pipeline).

Math: for each row (b,t), layer l attends over a small per-row source stack
(embedding, completed block sums S_k, running partial). Sources V = M @ X for
a constant 0/1 prefix matrix M (25x25) over the 25 raw per-row vectors
X = [emb, f_0..f_23]. Scores use rmsnorm'd keys: score[l,n] = rsq_n *
(v_n . qw_l) with qw = queries * key_norm_weight and rsq_n =
rsqrt(mean(v_n^2)+eps); h_l = softmax-weighted sum of sources = (A M) @ X.

Device pipeline per batch of R=5 rows (P = 125 partitions = (r, j)):
  1. SWDGE DMA X [125, 2048] bf16 (512KB) - gpsimd path sprays one transfer
     across all 16 SDMA engines (HWDGE chunks a big read onto ~5 engines).
  2. PE "fold-transposes": VT chunk = X_chunk.T @ M_bd  (the prefix fold M is
     streamed instead of the identity, so transpose+fold is one matmul);
     PSUM -> SBUF bf16 evac interleaved ACT/DVE, qwT copied into the gaps.
  3. PE SC' = VT.T @ [VT | qwT] accumulated over 16 d-chunks
     = [GramV | raw scores]; all matmul weights are 128-col bf16 (FWL).
  4. DVE eye-masked row-reduce of GramV -> sumsq; ACT rsqrt via exp(-.5*ln);
     scores scaled, transposed (PE), masked softmax over sources (DVE/ACT).
  5. alphas folded through M on PE (BT = M_bd.T @ abd); H = BT.T @ X in bf16,
     4x N=512; PSUM -> SBUF bf16, one contiguous 480KB store (scalar HWDGE).

Sharding: data-parallel over B*T = 2048 rows -> 8 cores x 256 rows.
I/O is bf16 end-to-end (rel-err budget 2e-2; measured ~2e-3): halves HBM
traffic vs fp32 - 26MB in + 25MB out per core.
"""

import numpy as np
import ml_dtypes

import concourse.bass as bass
import concourse.bacc as bacc
import concourse.mybir as mybir
from concourse import tile
from concourse.alu_op_type import AluOpType
from concourse.bass_utils import run_bass_kernel_spmd

L = 24
D = 2048
NUM_BLOCKS = 8
EPS = 1e-6
B, T = 2, 1024
N_CORES = 8

ROWS_PER_CORE = (B * T) // N_CORES  # 256
R = 5             # rows per batch
NJ = 25           # raw vectors per row: emb + 24 layer outputs
NS = 25           # sources per row
P = NJ * R        # 125 partitions per batch
NCHUNK = D // 128  # 16 d-chunks
CW = 152          # vt_sb per-chunk pitch: 128 (VT+3 zero pad) + 24 qwT
NEG = -1e30

f32 = mybir.dt.float32
bf16 = mybir.dt.bfloat16
BF = ml_dtypes.bfloat16


def _source_matrix():
    """M[n, j]: source n = sum_j M[n,j] * raw_j. Raw j=0 is emb, j=1+l is f_l."""
    M = np.zeros((NS, NJ), dtype=np.float32)
    M[0, 0] = 1.0
    for k in range(NUM_BLOCKS):
        for i in range(3):
            M[1 + 3 * k + i, 1 + 3 * k : 1 + 3 * k + i + 1] = 1.0
    return M


def _valid_matrix():
    """valid[l, n]: which sources layer l attends over."""
    V = np.zeros((L, NS), dtype=bool)
    for l in range(L):
        kb, ii = l // 3, l % 3
        V[l, 0] = True
        for k in range(kb):
            V[l, 3 * k + 3] = True
        if ii > 0:
            V[l, 3 * kb + ii] = True
    return V


def _build_consts(queries, key_norm_weight):
    M = _source_matrix()
    valid = _valid_matrix()
    eye_r = np.eye(R, dtype=np.float32)

    qw = (queries * key_norm_weight[None, :]).astype(np.float32)  # [L, D]
    # qwT[p, c*24 + l] = qw[l, c*128 + p]
    qwT = np.ascontiguousarray(
        qw.reshape(L, NCHUNK, 128).transpose(2, 1, 0).reshape(128, NCHUNK * L)
    ).astype(BF)

    # mtbd[(r,j),(r',n)] = (r==r') * M[n,j]; padded to 128 cols (zeros)
    mtbd = np.einsum("nj,ab->ajbn", M, eye_r).reshape(P, NS * R)
    mtbd128 = np.zeros((P, 128), np.float32)
    mtbd128[:, :P] = mtbd
    mtbd128 = mtbd128.astype(BF)
    # mbd[(r,n),(r',j)] = (r==r') * M[n,j]; padded to 128 cols
    mbd = np.einsum("nj,ab->anbj", M, eye_r).reshape(NS * R, P)
    mbd128 = np.zeros((P, 128), np.float32)
    mbd128[:, :P] = mbd
    mbd128 = mbd128.astype(BF)
    # eye mask for GramV diagonal extraction, padded to 128 cols
    eye_bd = np.zeros((P, 128), np.float32)
    eye_bd[:, :P] = np.eye(P, dtype=np.float32)
    # diagm[(r,n),(r',l)] = (r==r')
    diagm = np.einsum("ab,nl->anbl", eye_r, np.ones((NS, L), np.float32))
    diagm = np.ascontiguousarray(diagm.reshape(P, R * L)).astype(np.float32)
    # maskneg[l, (r,n)] = 0 if valid else NEG
    maskneg = np.where(valid[:, None, :], 0.0, NEG)
    maskneg = np.broadcast_to(maskneg, (L, R, NS)).reshape(L, R * NS)
    maskneg = np.ascontiguousarray(maskneg).astype(np.float32)

    ident = np.eye(128, dtype=np.float32)
    return dict(qwT=qwT, mtbd=mtbd128, mbd=mbd128, eyebd=eye_bd, diagm=diagm,
                maskneg=maskneg, ident=ident)


def _batch_starts():
    starts = [R * b for b in range(ROWS_PER_CORE // R)]  # 0..250
    if starts[-1] + R < ROWS_PER_CORE:
        starts.append(ROWS_PER_CORE - R)  # 251 (overlaps; identical rewrites)
    return starts


def build_kernel():
    nc = bacc.Bacc("TRN2", target_bir_lowering=False, debug=False)

    # bf16 row-major input [row, j, d] flattened, fully contiguous.
    loT = nc.dram_tensor("loT", [ROWS_PER_CORE * NJ, D], bf16,
                         kind="ExternalInput").ap()
    qwT_d = nc.dram_tensor("qwT", [128, NCHUNK * L], bf16, kind="ExternalInput").ap()
    mtbd_d = nc.dram_tensor("mtbd", [P, 128], bf16, kind="ExternalInput").ap()
    mbd_d = nc.dram_tensor("mbd", [P, 128], bf16, kind="ExternalInput").ap()
    eyebd_d = nc.dram_tensor("eyebd", [P, 128], f32, kind="ExternalInput").ap()
    diagm_d = nc.dram_tensor("diagm", [P, R * L], f32, kind="ExternalInput").ap()
    maskneg_d = nc.dram_tensor("maskneg", [L, R * NS], f32, kind="ExternalInput").ap()
    ident_d = nc.dram_tensor("ident", [128, 128], f32, kind="ExternalInput").ap()
    # bf16 output [row, l, d] flattened; host casts + transposes back
    outT = nc.dram_tensor("outT", [ROWS_PER_CORE * L, D], bf16,
                          kind="ExternalOutput").ap()

    with tile.TileContext(nc) as tc:
        with (
            tc.tile_pool(name="const", bufs=1) as const,
            tc.tile_pool(name="xpool", bufs=6) as xpool,
            tc.tile_pool(name="vtpool", bufs=3) as vtpool,
            tc.tile_pool(name="hpool", bufs=3) as hpool,
            tc.tile_pool(name="small", bufs=2) as small,
            tc.tile_pool(name="ps_ft", bufs=2, space=bass.MemorySpace.PSUM) as ps_ft,
            tc.tile_pool(name="ps_sc", bufs=2, space=bass.MemorySpace.PSUM) as ps_sc,
            tc.tile_pool(name="ps_sm", bufs=2, space=bass.MemorySpace.PSUM) as ps_sm,
            tc.tile_pool(name="ps_h", bufs=2, space=bass.MemorySpace.PSUM) as ps_h,
        ):
            qwT = const.tile([128, NCHUNK * L], bf16)
            nc.sync.dma_start(qwT[:], qwT_d[:])
            mtbd = const.tile([P, 128], bf16)
            nc.sync.dma_start(mtbd[:], mtbd_d[:])
            mbd = const.tile([P, 128], bf16)
            nc.sync.dma_start(mbd[:], mbd_d[:])
            eyebd = const.tile([P, 128], f32)
            nc.sync.dma_start(eyebd[:], eyebd_d[:])
            diagm = const.tile([P, R * L], f32)
            nc.sync.dma_start(diagm[:], diagm_d[:])
            maskneg = const.tile([L, R * NS], f32)
            nc.sync.dma_start(maskneg[:], maskneg_d[:])
            ident = const.tile([128, 128], f32)
            nc.sync.dma_start(ident[:], ident_d[:])
            epsb = const.tile([P, 1], f32)
            nc.vector.memset(epsb[:], EPS)

            for row0 in _batch_starts():
                # ---- X = [emb; f_0..f_23] per row: one 512KB SWDGE DMA
                X = xpool.tile([P, D], bf16)
                nc.gpsimd.dma_start(X[:], loT[row0 * NJ : row0 * NJ + P, :])

                # ---- fold-transposes: VT chunk = X_chunk.T @ M_bd (PSUM f32)
                vt_sb = vtpool.tile([128, NCHUNK * CW], bf16)
                vt3 = vt_sb.rearrange("p (c w) -> p c w", w=CW)
                nc.scalar.copy(
                    vt3[:, :, 128 : 128 + L],
                    qwT.rearrange("p (c w) -> p c w", w=L),
                )
                for half in range(4):
                    ftp = ps_ft.tile([128, 512], f32)
                    for cc in range(4):
                        c = 4 * half + cc
                        nc.tensor.matmul(
                            ftp[:, 128 * cc : 128 * (cc + 1)],
                            X[:, 128 * c : 128 * (c + 1)],
                            mtbd[:],
                            start=True,
                            stop=True,
                        )
                    ft4 = ftp.rearrange("p (cc w) -> p cc w", w=128)
                    dst = vt3[:, 4 * half : 4 * half + 4, 0:128]
                    if half % 2 == 0:
                        nc.scalar.copy(dst, ft4)
                    else:
                        nc.vector.tensor_copy(dst, ft4)

                # ---- SC' = [GramV | raw scores] accumulated over d-chunks
                SCp = ps_sc.tile([128, CW], f32)
                for c in range(NCHUNK):
                    nc.tensor.matmul(
                        SCp[:],
                        vt3[:, c, 0:128],
                        vt3[:, c, 0:CW],
                        start=(c == 0),
                        stop=(c == NCHUNK - 1),
                    )

                # ---- sumsq_n = GramV[n, n] via eye-masked row reduce
                junk = small.tile([P, 128], f32)
                sumsq = small.tile([P, 1], f32)
                nc.vector.scalar_tensor_tensor(
                    out=junk[:],
                    in0=SCp[0:P, 0:128],
                    scalar=1.0,
                    in1=eyebd[:],
                    op0=AluOpType.mult,
                    op1=AluOpType.mult,
                    accum_out=sumsq[:],
                )
                # rsqrt(mean+eps) = exp(-0.5 * ln(sumsq/D + eps))
                lnu = small.tile([P, 1], f32)
                nc.scalar.activation(
                    lnu[:], sumsq[:], mybir.ActivationFunctionType.Ln,
                    bias=epsb[:], scale=1.0 / D,
                )
                rsq = small.tile([P, 1], f32)
                nc.scalar.activation(
                    rsq[:], lnu[:], mybir.ActivationFunctionType.Exp, scale=-0.5
                )
                scoresR = small.tile([P, L], f32)
                nc.scalar.activation(
                    scoresR[:], SCp[0:P, 128 : 128 + L],
                    mybir.ActivationFunctionType.Copy, scale=rsq[:],
                )

                # ---- masked softmax over sources (free axis), per (r, l)
                scoreT = ps_sm.tile([L, P], f32, tag="sm")
                nc.tensor.transpose(scoreT[:], scoresR[:], ident[:P, :P])
                smask = small.tile([L, P], f32)
                nc.vector.tensor_add(smask[:], scoreT[:], maskneg[:])
                esc = small.tile([L, P], f32)
                nc.scalar.activation(
                    esc[:], smask[:], mybir.ActivationFunctionType.Exp
                )
                ssum = small.tile([L, R], f32)
                nc.vector.reduce_sum(
                    ssum[:],
                    esc.rearrange("p (r n) -> p r n", r=R),
                    axis=mybir.AxisListType.X,
                )
                rec = small.tile([L, R], f32)
                nc.vector.reciprocal(rec[:], ssum[:])
                alpha = small.tile([L, P], f32)
                nc.vector.tensor_tensor(
                    alpha.rearrange("p (r n) -> p r n", r=R),
                    esc.rearrange("p (r n) -> p r n", r=R),
                    rec.unsqueeze(2).broadcast_to([L, R, NS]),
                    AluOpType.mult,
                )

                # ---- fold alphas through M: BT = M_bd.T @ abd
                alphaT = ps_sm.tile([P, L], f32, tag="sm")
                nc.tensor.transpose(alphaT[:], alpha[:], ident[:L, :L])
                abd = small.tile([P, 128], bf16)
                nc.vector.memset(abd[:, 120:128], 0.0)
                nc.vector.scalar_tensor_tensor(
                    out=abd[:, 0:120].rearrange("p (r l) -> p r l", r=R),
                    in0=alphaT.unsqueeze(1).broadcast_to([P, R, L]),
                    scalar=1.0,
                    in1=diagm.rearrange("p (r l) -> p r l", r=R),
                    op0=AluOpType.mult,
                    op1=AluOpType.mult,
                )
                BTp = ps_sm.tile([128, 128], f32, tag="sm")
                nc.tensor.matmul(BTp[:], mbd[:], abd[:], start=True, stop=True)
                btsb = small.tile([128, 128], bf16)
                nc.scalar.copy(btsb[:], BTp[:])

                # ---- H = BT.T @ X (bf16, N=512 per PSUM bank)
                h_sb = hpool.tile([R * L, D], bf16)
                for nb in range(4):
                    Hp = ps_h.tile([128, 512], f32)
                    nc.tensor.matmul(
                        Hp[:],
                        btsb[0:P, :],
                        X[:, 512 * nb : 512 * (nb + 1)],
                        start=True,
                        stop=True,
                    )
                    if nb % 2 == 0:
                        nc.scalar.copy(h_sb[:, 512 * nb : 512 * (nb + 1)],
                                       Hp[0 : R * L, :])
                    else:
                        nc.vector.tensor_copy(h_sb[:, 512 * nb : 512 * (nb + 1)],
                                              Hp[0 : R * L, :])

                # out-DMA on the ACT HWDGE ring (contiguous 480KB bf16)
                nc.scalar.dma_start(
                    outT[row0 * L : row0 * L + R * L, :], h_sb[:]
                )

    # Pin Ln/Exp to the one table set containing both -> single ACT table load.
    real_gat = bacc.get_activation_tables
    AF = mybir.ActivationFunctionType

    def gat_pinned(arch):
        out = {}
        for name, fns in real_gat(arch).items():
            if name == "natural_log_exp_and_others":
                out[name] = set(fns)
            else:
                out[name] = {f for f in fns if f not in (AF.Ln, AF.Exp)}
        return out

    bacc.get_activation_tables = gat_pinned
    try:
        nc.compile()
    finally:
        bacc.get_activation_tables = real_gat
    return nc


_NC_CACHE = None


def _prep_loT(layer_outputs, embedding):
    """[L,B,T,D]+[B,T,D] -> bf16 per-row stacks [B*T, 25, D] (row-major)."""
    loT = np.empty((B * T, NJ, D), dtype=BF)
    loT[:, 0, :] = embedding.reshape(B * T, D).astype(BF)
    loT[:, 1:, :] = (
        layer_outputs.reshape(L, B * T, D).transpose(1, 0, 2).astype(BF)
    )
    return loT


def _make_in_maps(layer_outputs, embedding, queries, key_norm_weight):
    loT = _prep_loT(layer_outputs, embedding)
    consts = _build_consts(queries, key_norm_weight)
    in_maps = []
    for c in range(N_CORES):
        r0 = c * ROWS_PER_CORE
        in_maps.append({
            "loT": loT[r0 : r0 + ROWS_PER_CORE].reshape(ROWS_PER_CORE * NJ, D),
            "qwT": consts["qwT"],
            "mtbd": consts["mtbd"],
            "mbd": consts["mbd"],
            "eyebd": consts["eyebd"],
            "diagm": consts["diagm"],
            "maskneg": consts["maskneg"],
            "ident": consts["ident"],
        })
    return in_maps


def kernel(layer_outputs, embedding, queries, key_norm_weight):
    global _NC_CACHE
    layer_outputs = np.asarray(layer_outputs, dtype=np.float32)
    embedding = np.asarray(embedding, dtype=np.float32)
    queries = np.asarray(queries, dtype=np.float32)
    key_norm_weight = np.asarray(key_norm_weight, dtype=np.float32)

    in_maps = _make_in_maps(layer_outputs, embedding, queries, key_norm_weight)

    if _NC_CACHE is None:
        _NC_CACHE = build_kernel()
    nc = _NC_CACHE

    res = run_bass_kernel_spmd(nc, in_maps, core_ids=list(range(N_CORES)))

    full = np.empty((L, B * T, D), dtype=np.float32)
    for c in range(N_CORES):
        r0 = c * ROWS_PER_CORE
        outT = res.results[c]["outT"].astype(np.float32).reshape(
            ROWS_PER_CORE, L, D
        )
        full[:, r0 : r0 + ROWS_PER_CORE, :] = outT.transpose(1, 0, 2)
    return full.reshape(L, B, T, D)
